# revision 1
# baseline (speedup 1.0000x reference)
"""GINEConv layer (gather -> relu(x_src+ea) -> segment_sum -> MLP -> residual LN)
as a Bass/Tile kernel on 8 TRN2 NeuronCores.

Sharding: nodes are block-partitioned across cores (6250/core); edges are
partitioned by destination owner and sorted by dst; x is replicated in HBM and
gathered on-device by src index (indirect DMA); edge_attr rows are
host-permuted into per-core dst-sorted order (sequential DMA). Each core
computes its node shard's full pipeline independently; the host reassembles.
"""
import sys
sys.path.insert(0, "/opt/trn_rl_repo")
from contextlib import ExitStack

import numpy as np

import concourse.bass as bass
import concourse.tile as tile
from concourse import bacc, mybir
from concourse.bass_utils import run_bass_kernel_spmd
from concourse.masks import make_identity

P = 128
H = 512
H4 = 2048
NC_ = 8
N = 50000
E = 150000
NLOC = N // NC_            # 6250 nodes per core
NBLK = 52                  # 128-node blocks per core (52*128 = 6656 >= 6250)
NLOCP = NBLK * P           # padded per-core node count
SB = 4                     # blocks per super-block
NSB = NBLK // SB           # 13 super-blocks
FC = H // P                # 4 feature chunks
F2C = H4 // P              # 16 hidden chunks
LN_EPS = 1e-5
OOB = 1 << 30              # pad src index => descriptor skipped via bounds check

F32 = mybir.dt.float32
F32R = mybir.dt.float32r
I32 = mybir.dt.int32
AF = mybir.ActivationFunctionType
OP = mybir.AluOpType


def _build_program(TB, apply_gamma_beta):
    nc = bacc.Bacc("TRN2", target_bir_lowering=False, num_devices=NC_)

    xfull = nc.declare_dram_parameter("xfull", [N, H], F32, isOutput=False)
    xloc = nc.declare_dram_parameter("xloc", [NLOCP, H], F32, isOutput=False)
    xtloc = nc.declare_dram_parameter("xtloc", [H, NLOCP], F32, isOutput=False)
    eaperm = nc.declare_dram_parameter("eaperm", [NBLK * TB * P, H], F32, isOutput=False)
    srcidx = nc.declare_dram_parameter("srcidx", [NBLK, P, TB], I32, isOutput=False)
    dstloc = nc.declare_dram_parameter("dstloc", [NBLK, P, TB], F32, isOutput=False)
    w1 = nc.declare_dram_parameter("w1", [H, H4], F32, isOutput=False)
    w2 = nc.declare_dram_parameter("w2", [H4, H], F32, isOutput=False)
    gbt = nc.declare_dram_parameter("gbt", [P, 2 * FC], F32, isOutput=False)
    iota_in = nc.declare_dram_parameter("iota", [P, P], F32, isOutput=False)
    outT = nc.declare_dram_parameter("outT", [H, NLOCP], F32, isOutput=True)

    with tile.TileContext(nc) as tc, ExitStack() as ctx:
        keep = ctx.enter_context(tc.tile_pool(name="keep", bufs=1))
        idxp = ctx.enter_context(tc.tile_pool(name="idxp", bufs=3))
        gats = ctx.enter_context(tc.tile_pool(name="gats", bufs=8))
        eap = ctx.enter_context(tc.tile_pool(name="eap", bufs=8))
        msgp = ctx.enter_context(tc.tile_pool(name="msgp", bufs=6))
        selp = ctx.enter_context(tc.tile_pool(name="selp", bufs=6))
        xlp = ctx.enter_context(tc.tile_pool(name="xlp", bufs=2))
        hp = ctx.enter_context(tc.tile_pool(name="hp", bufs=2))
        htp = ctx.enter_context(tc.tile_pool(name="htp", bufs=2))
        gtp = ctx.enter_context(tc.tile_pool(name="gtp", bufs=17))
        xtp = ctx.enter_context(tc.tile_pool(name="xtp", bufs=2))
        tp = ctx.enter_context(tc.tile_pool(name="tp", bufs=5))
        sqp = ctx.enter_context(tc.tile_pool(name="sqp", bufs=2))
        stp = ctx.enter_context(tc.tile_pool(name="stp", bufs=2))
        outp = ctx.enter_context(tc.tile_pool(name="outp", bufs=2))
        # PSUM: 2 (aggr/transpose shared) + 2 (z) + 2 (y) + 2 (stats) = 8 banks
        pap = ctx.enter_context(tc.tile_pool(name="pap", bufs=3, space="PSUM"))
        pzp = ctx.enter_context(tc.tile_pool(name="pzp", bufs=2, space="PSUM"))
        pyp = ctx.enter_context(tc.tile_pool(name="pyp", bufs=2, space="PSUM"))
        psp = ctx.enter_context(tc.tile_pool(name="psp", bufs=1, space="PSUM"))

        # ---- constants / weights ----
        iota_sb = keep.tile([P, P], F32)
        nc.sync.dma_start(out=iota_sb[:], in_=iota_in[:])
        ident_f = keep.tile([P, P], F32)
        make_identity(nc, ident_f[:])
        ident = keep.tile([P, P], F32R)
        nc.scalar.activation(out=ident[:], in_=ident_f[:], func=AF.Copy)
        ones_f = keep.tile([P, 1], F32)
        nc.vector.memset(ones_f[:], 1.0)
        ones_sb = keep.tile([P, P], F32R)
        nc.scalar.activation(out=ones_sb[:], in_=ones_f[:].to_broadcast([P, P]), func=AF.Copy)
        eps_sb = keep.tile([P, 1], F32)
        nc.vector.memset(eps_sb[:], LN_EPS)
        gbt_sb = keep.tile([P, 2 * FC], F32)
        nc.sync.dma_start(out=gbt_sb[:], in_=gbt[:])

        # weights: DMA into F32 staging (reusing idle gt slots), round via ACT
        w1r = []
        for kc in range(FC):
            w = keep.tile([P, H4], F32R, tag=f"w1_{kc}", name=f"w1r_{kc}")
            for q in range(H4 // H):
                stg = gtp.tile([P, H], F32, tag="gt", name=f"wstg1_{kc}_{q}")
                nc.sync.dma_start(out=stg[:], in_=w1[kc * P : (kc + 1) * P, q * H : (q + 1) * H])
                nc.scalar.activation(out=w[:, q * H : (q + 1) * H], in_=stg[:], func=AF.Copy)
            w1r.append(w)
        w2r = []
        for kc in range(F2C):
            w = keep.tile([P, H], F32R, tag=f"w2_{kc}", name=f"w2r_{kc}")
            stg = gtp.tile([P, H], F32, tag="gt", name=f"wstg2_{kc}")
            nc.sync.dma_start(out=stg[:], in_=w2[kc * P : (kc + 1) * P, :])
            nc.scalar.activation(out=w[:], in_=stg[:], func=AF.Copy)
            w2r.append(w)

        inv_h = 1.0 / H

        for sb in range(NSB):
            # ---------- phase A: aggregation for the 4 blocks ----------
            hts = [htp.tile([P, H], F32R, tag=f"ht{fc}", name=f"ht{fc}_{sb}") for fc in range(FC)]
            for b in range(SB):
                j = sb * SB + b
                sidx = idxp.tile([P, TB], I32, tag="sidx")
                nc.sync.dma_start(out=sidx[:], in_=srcidx[j])
                dloc = idxp.tile([P, TB], F32, tag="dloc")
                nc.sync.dma_start(out=dloc[:], in_=dstloc[j])
                pa = pap.tile([P, H], F32, tag="pa")
                for t in range(TB):
                    xs = gats.tile([P, H], F32)
                    nc.gpsimd.indirect_dma_start(
                        out=xs[:], out_offset=None, in_=xfull[:],
                        in_offset=bass.IndirectOffsetOnAxis(ap=sidx[:, t : t + 1], axis=0),
                        bounds_check=N - 1, oob_is_err=False,
                    )
                    ea = eap.tile([P, H], F32)
                    row0 = (j * TB + t) * P
                    nc.sync.dma_start(out=ea[:], in_=eaperm[row0 : row0 + P, :])
                    msg = msgp.tile([P, H], F32R)
                    nc.vector.tensor_tensor(out=msg[:], in0=xs[:], in1=ea[:], op=OP.add)
                    nc.scalar.activation(out=msg[:], in_=msg[:], func=AF.Relu)
                    sel = selp.tile([P, P], F32R)
                    nc.vector.tensor_tensor(
                        out=sel[:],
                        in0=dloc[:, t : t + 1].to_broadcast([P, P]),
                        in1=iota_sb[:],
                        op=OP.is_equal,
                    )
                    nc.tensor.matmul(out=pa[:], lhsT=sel[:], rhs=msg[:],
                                     start=(t == 0), stop=(t == TB - 1))
                # h = x + aggr, then transpose h into hts[fc][:, b*128:...]
                xl = xlp.tile([P, H], F32)
                nc.sync.dma_start(out=xl[:], in_=xloc[j * P : (j + 1) * P, :])
                h = hp.tile([P, H], F32R)
                nc.vector.tensor_tensor(out=h[:], in0=xl[:], in1=pa[:], op=OP.add)
                for fc in range(FC):
                    ptr = pap.tile([P, P], F32R, tag="pa")
                    nc.tensor.transpose(out=ptr[:], in_=h[:, fc * P : (fc + 1) * P],
                                        identity=ident[:])
                    nc.scalar.activation(out=hts[fc][:, b * P : (b + 1) * P],
                                         in_=ptr[:], func=AF.Copy)

            # ---------- phase B: MLP1 (h @ W1), gelu ----------
            gts = []
            for f2c in range(F2C):
                pz = pzp.tile([P, H], F32, tag="pz")
                for kc in range(FC):
                    nc.tensor.matmul(
                        out=pz[:], lhsT=w1r[kc][:, f2c * P : (f2c + 1) * P],
                        rhs=hts[kc][:], start=(kc == 0), stop=(kc == FC - 1))
                gt = gtp.tile([P, H], F32R, tag="gt")
                nc.scalar.activation(out=gt[:], in_=pz[:], func=AF.Gelu)
                gts.append(gt)

            # ---------- phase C: MLP2 (g @ W2) + residual ----------
            ts = []
            for fc in range(FC):
                py = pyp.tile([P, H], F32, tag="py")
                for kc in range(F2C):
                    nc.tensor.matmul(
                        out=py[:], lhsT=w2r[kc][:, fc * P : (fc + 1) * P],
                        rhs=gts[kc][:], start=(kc == 0), stop=(kc == F2C - 1))
                xt = xtp.tile([P, H], F32, tag="xt")
                nc.sync.dma_start(
                    out=xt[:],
                    in_=xtloc[fc * P : (fc + 1) * P, sb * SB * P : (sb + 1) * SB * P])
                t_ = tp.tile([P, H], F32R, tag="t")
                nc.vector.tensor_tensor(out=t_[:], in0=xt[:], in1=py[:], op=OP.add)
                ts.append(t_)

            # ---------- phase D: LayerNorm over features (partition axis) ----------
            psum_s = psp.tile([P, H], F32, tag="ps")
            for fc in range(FC):
                nc.tensor.matmul(out=psum_s[:], lhsT=ones_sb[:], rhs=ts[fc][:],
                                 start=(fc == 0), stop=(fc == FC - 1))
            mean = stp.tile([P, H], F32, tag="mean", bufs=1)
            nc.vector.tensor_scalar_mul(out=mean[:], in0=psum_s[:], scalar1=inv_h)
            psum_q = psp.tile([P, H], F32, tag="ps")
            sqs = []
            for fc in range(FC):
                sq = sqp.tile([P, H], F32R, tag="sq")
                nc.scalar.activation(out=sq[:], in_=ts[fc][:], func=AF.Square)
                sqs.append(sq)
                nc.tensor.matmul(out=psum_q[:], lhsT=ones_sb[:], rhs=sq[:],
                                 start=(fc == 0), stop=(fc == FC - 1))
            msq = stp.tile([P, H], F32, tag="tmp")
            nc.scalar.activation(out=msq[:], in_=mean[:], func=AF.Square)
            var = stp.tile([P, H], F32, tag="tmp")
            nc.vector.scalar_tensor_tensor(
                out=var[:], in0=psum_q[:], scalar=inv_h, in1=msq[:],
                op0=OP.mult, op1=OP.subtract)
            std = stp.tile([P, H], F32, tag="tmp")
            nc.scalar.activation(out=std[:], in_=var[:], func=AF.Sqrt, bias=eps_sb[:])
            rstd = stp.tile([P, H], F32, tag="tmp")
            nc.vector.reciprocal(out=rstd[:], in_=std[:])
            for fc in range(FC):
                u = outp.tile([P, H], F32, tag="u")
                nc.vector.tensor_tensor(out=u[:], in0=ts[fc][:], in1=mean[:],
                                        op=OP.subtract)
                o = outp.tile([P, H], F32, tag="o")
                if apply_gamma_beta:
                    nc.vector.scalar_tensor_tensor(
                        out=o[:], in0=u[:], scalar=gbt_sb[:, fc : fc + 1],
                        in1=rstd[:], op0=OP.mult, op1=OP.mult)
                    nc.vector.tensor_scalar_add(
                        out=o[:], in0=o[:], scalar1=gbt_sb[:, FC + fc : FC + fc + 1])
                else:
                    nc.vector.tensor_tensor(out=o[:], in0=u[:], in1=rstd[:],
                                            op=OP.mult)
                nc.sync.dma_start(
                    out=outT[fc * P : (fc + 1) * P, sb * SB * P : (sb + 1) * SB * P],
                    in_=o[:])

    nc.compile()
    return nc


def _prep(x, edge_attr, W1, W2, gamma, beta, edge_index):
    src = np.asarray(edge_index[0], dtype=np.int64)
    dst = np.asarray(edge_index[1], dtype=np.int64)
    x = np.ascontiguousarray(np.asarray(x, dtype=np.float32))
    edge_attr = np.ascontiguousarray(np.asarray(edge_attr, dtype=np.float32))

    owner = dst // NLOC
    order = np.argsort(owner * N + dst, kind="stable")
    src_s, dst_s, eid_s = src[order], dst[order], order

    # per (core, block) counts
    blk_of = (dst_s - (owner[order] * NLOC)) // P  # local block id
    core_of = owner[order]
    counts = np.zeros((NC_, NBLK), dtype=np.int64)
    np.add.at(counts, (core_of, blk_of), 1)
    TB = max(1, int(np.ceil(counts.max() / P)))

    in_maps = []
    slots = NBLK * TB * P
    # boundaries of each (core, block) run in the sorted edge list
    run_starts = np.zeros((NC_, NBLK), dtype=np.int64)
    flat_counts = counts.reshape(-1)
    run_starts.reshape(-1)[1:] = np.cumsum(flat_counts)[:-1]

    for c in range(NC_):
        n0 = c * NLOC
        srcidx = np.full((NBLK, P, TB), OOB, dtype=np.int32)
        dstloc = np.full((NBLK, P, TB), -1.0, dtype=np.float32)
        eaid = np.zeros((NBLK, TB * P), dtype=np.int64)
        eamask = np.zeros((NBLK, TB * P), dtype=bool)
        for j in range(NBLK):
            cnt = int(counts[c, j])
            if cnt == 0:
                continue
            s0 = int(run_starts[c, j])
            sl = slice(s0, s0 + cnt)
            kk = np.arange(cnt)
            t_i, p_i = kk // P, kk % P
            srcidx[j, p_i, t_i] = src_s[sl].astype(np.int32)
            dstloc[j, p_i, t_i] = (dst_s[sl] - n0 - j * P).astype(np.float32)
            eaid[j, :cnt] = eid_s[sl]
            eamask[j, :cnt] = True
        # pads in the first 2 blocks must gather a real row (pool bufs warmup)
        for j in range(2):
            blk = srcidx[j]
            blk[blk == OOB] = 0
        eaperm = np.zeros((slots, H), dtype=np.float32)
        flat_ids = eaid.reshape(-1)
        flat_mask = eamask.reshape(-1)
        eaperm[flat_mask] = edge_attr[flat_ids[flat_mask]]

        xloc = np.zeros((NLOCP, H), dtype=np.float32)
        xloc[:NLOC] = x[n0 : n0 + NLOC]
        xtloc = np.ascontiguousarray(xloc.T)

        gbt = np.zeros((P, 2 * FC), dtype=np.float32)
        gbt[:, :FC] = np.asarray(gamma, dtype=np.float32).reshape(FC, P).T
        gbt[:, FC:] = np.asarray(beta, dtype=np.float32).reshape(FC, P).T
        iota = np.broadcast_to(np.arange(P, dtype=np.float32), (P, P)).copy()

        in_maps.append({
            "xfull": x, "xloc": xloc, "xtloc": xtloc, "eaperm": eaperm,
            "srcidx": srcidx, "dstloc": dstloc,
            "w1": np.ascontiguousarray(np.asarray(W1, dtype=np.float32)),
            "w2": np.ascontiguousarray(np.asarray(W2, dtype=np.float32)),
            "gbt": gbt, "iota": iota,
        })
    return in_maps, TB


LAST_EXEC_NS = None


def kernel(x, edge_attr, W1, W2, gamma, beta, edge_index):
    global LAST_EXEC_NS
    in_maps, TB = _prep(x, edge_attr, W1, W2, gamma, beta, edge_index)
    gamma_np = np.asarray(gamma, dtype=np.float32)
    beta_np = np.asarray(beta, dtype=np.float32)
    apply_gb = not (np.all(gamma_np == 1.0) and np.all(beta_np == 0.0))
    nc = _build_program(TB, apply_gb)
    try:
        from concourse.timeline_sim import TimelineSim
        LAST_EXEC_NS = int(TimelineSim(nc, trace=False).simulate())
    except Exception:
        LAST_EXEC_NS = None
    rr = run_bass_kernel_spmd(nc, in_maps, list(range(NC_)))
    if rr.exec_time_ns is not None:
        LAST_EXEC_NS = int(rr.exec_time_ns)
    res = rr.results
    out = np.empty((N, H), dtype=np.float32)
    for c in range(NC_):
        out[c * NLOC : (c + 1) * NLOC] = res[c]["outT"][:, :NLOC].T
    return out



# revision 9
# speedup vs baseline: 1.4233x; 1.4233x over previous
"""GINEConv layer (gather -> relu(x_src+ea) -> segment_sum -> MLP -> residual LN)
as a Bass/Tile kernel on 8 TRN2 NeuronCores.

Sharding: nodes block-partitioned across cores (6250/core, 49 blocks of 128);
edges partitioned by destination owner and sorted by dst. The whole data path
runs in bf16 (PE matmul rate is identical to fp32r, but DMA/DVE cost halves).
The segment-sum scatter is computed feature-major (out[feat, node] = msg^T @
one_hot) so the aggregated h lands pre-transposed for the MLP — no PE
transposes and no PSUM->SBUF copies. Per-block DMAs are batched (one gather,
one edge-attr load, one index load per block). Super-blocks of 4 blocks
(plus one runt block) are software-pipelined: phase A of super-block k+1 is
emitted before the MLP/LN of super-block k so DVE/DMA prefetch runs under the
MLP matmuls.
"""
import os
import sys
sys.path.insert(0, "/opt/trn_rl_repo")
from contextlib import ExitStack

import numpy as np
import ml_dtypes

import concourse.bass as bass
import concourse.tile as tile
from concourse import bacc, mybir
from concourse.bass_utils import run_bass_kernel_spmd

P = 128
H = 512
H4 = 2048
NC_ = 8
N = 50000
E = 150000
NLOC = N // NC_            # 6250 nodes per core
NBLK = 49                  # 128-node blocks per core (49*128 = 6272 >= 6250)
NLOCP = NBLK * P
FC = H // P                # 4 feature chunks
F2C = H4 // P              # 16 hidden chunks
LN_EPS = 1e-5
OOB = 1 << 30              # pad src index => descriptor skipped via bounds check
WARM = 6                   # first-N blocks: pads must gather a real row (SBUF warmup)

# super-blocks: 12 x 4 blocks + 1 x 1 block (runt)
SBS = [(i * 4, 4) for i in range(12)] + [(48, 1)]
XCOLS = sum(FC * nb * P for _, nb in SBS)  # 12*2048 + 512 = 25088
SBOFF = np.cumsum([0] + [FC * nb * P for _, nb in SBS]).tolist()

F32 = mybir.dt.float32
BF16 = mybir.dt.bfloat16
I32 = mybir.dt.int32
AF = mybir.ActivationFunctionType
OP = mybir.AluOpType


def _build_program(TB, apply_gamma_beta):
    nc = bacc.Bacc("TRN2", target_bir_lowering=False, num_devices=NC_)

    xfull = nc.declare_dram_parameter("xfull", [N, H], BF16, isOutput=False)
    idxcomb = nc.declare_dram_parameter("idxcomb", [NBLK, P, 2 * TB], I32, isOutput=False)
    eapack = nc.declare_dram_parameter("eapack", [NBLK, P, TB * H], BF16, isOutput=False)
    xtpack = nc.declare_dram_parameter("xtpack", [P, XCOLS], BF16, isOutput=False)
    w1p = nc.declare_dram_parameter("w1p", [P, FC * H4], BF16, isOutput=False)
    w2p = nc.declare_dram_parameter("w2p", [P, F2C * H], BF16, isOutput=False)
    iota_in = nc.declare_dram_parameter("iota", [P, P], I32, isOutput=False)
    gbt = nc.declare_dram_parameter("gbt", [P, 2 * FC], F32, isOutput=False)
    outD = nc.declare_dram_parameter("outD", [P, XCOLS], BF16, isOutput=True)
    dbg = os.environ.get("KBD_DEBUG") == "1"
    if dbg:
        dbg_xs = nc.declare_dram_parameter("dbg_xs", [P, TB * H], BF16, isOutput=True)
        dbg_sel = nc.declare_dram_parameter("dbg_sel", [TB, P, P], BF16, isOutput=True)
        dbg_pa = nc.declare_dram_parameter("dbg_pa", [P, FC * P], F32, isOutput=True)
        dbg_ht = nc.declare_dram_parameter("dbg_ht", [P, FC * 4 * P], BF16, isOutput=True)

    with tile.TileContext(nc) as tc, ExitStack() as ctx:
        keep = ctx.enter_context(tc.tile_pool(name="keep", bufs=1))
        idxp = ctx.enter_context(tc.tile_pool(name="idxp", bufs=4))
        gats = ctx.enter_context(tc.tile_pool(name="gats", bufs=6))
        eap = ctx.enter_context(tc.tile_pool(name="eap", bufs=6))
        msgp = ctx.enter_context(tc.tile_pool(name="msgp", bufs=3))
        selp = ctx.enter_context(tc.tile_pool(name="selp", bufs=8))
        htp = ctx.enter_context(tc.tile_pool(name="htp", bufs=2))
        gtp = ctx.enter_context(tc.tile_pool(name="gtp", bufs=2))
        xtp = ctx.enter_context(tc.tile_pool(name="xtp", bufs=2))
        tsp = ctx.enter_context(tc.tile_pool(name="tsp", bufs=2))
        sqp = ctx.enter_context(tc.tile_pool(name="sqp", bufs=2))
        stp = ctx.enter_context(tc.tile_pool(name="stp", bufs=2))
        up = ctx.enter_context(tc.tile_pool(name="up", bufs=2))
        outp = ctx.enter_context(tc.tile_pool(name="outp", bufs=2))
        # PSUM: 2 (scatter) + 2 (z) + 2 (y) + 2 (stats) = 8 banks
        pap = ctx.enter_context(tc.tile_pool(name="pap", bufs=2, space="PSUM"))
        pzp = ctx.enter_context(tc.tile_pool(name="pzp", bufs=2, space="PSUM"))
        pyp = ctx.enter_context(tc.tile_pool(name="pyp", bufs=2, space="PSUM"))
        psp = ctx.enter_context(tc.tile_pool(name="psp", bufs=2, space="PSUM"))

        # ---- constants / weights ----
        iota_sb = keep.tile([P, P], I32)
        nc.sync.dma_start(out=iota_sb[:], in_=iota_in[:])
        ones_sb = keep.tile([P, P], BF16)
        nc.vector.memset(ones_sb[:], 1.0)
        eps_sb = keep.tile([P, 1], F32)
        nc.vector.memset(eps_sb[:], LN_EPS)
        gbt_sb = keep.tile([P, 2 * FC], F32)
        nc.sync.dma_start(out=gbt_sb[:], in_=gbt[:])
        w1s = keep.tile([P, FC * H4], BF16)
        nc.sync.dma_start(out=w1s[:], in_=w1p[:])
        w2s = keep.tile([P, F2C * H], BF16)
        nc.sync.dma_start(out=w2s[:], in_=w2p[:])

        inv_h = 1.0 / H
        state = {}

        def phase_a(si):
            b0, nb = SBS[si]
            W = nb * P
            O = SBOFF[si]
            xt = xtp.tile([P, FC * 4 * P], BF16, tag="xt")
            nc.sync.dma_start(out=xt[:, : FC * W], in_=xtpack[:, O : O + FC * W])
            ht = htp.tile([P, FC * 4 * P], BF16, tag="ht")
            for b in range(nb):
                j = b0 + b
                idx = idxp.tile([P, 2 * TB], I32, tag="idx")
                nc.sync.dma_start(out=idx[:], in_=idxcomb[j])
                xs = gats.tile([P, TB * H], BF16, tag="xs")
                for t in range(TB):
                    nc.gpsimd.indirect_dma_start(
                        out=xs[:, t * H : (t + 1) * H], out_offset=None, in_=xfull[:],
                        in_offset=bass.IndirectOffsetOnAxis(ap=idx[:, t : t + 1], axis=0),
                        bounds_check=N - 1, oob_is_err=False,
                    )
                ea = eap.tile([P, TB * H], BF16, tag="ea")
                nc.sync.dma_start(out=ea[:], in_=eapack[j])
                msg = msgp.tile([P, TB * H], BF16, tag="msg")
                nc.vector.tensor_tensor(out=msg[:], in0=xs[:], in1=ea[:], op=OP.add)
                nc.scalar.activation(out=msg[:], in_=msg[:], func=AF.Relu)
                pa = pap.tile([P, FC * P], F32, tag="pa")
                sels = []
                for t in range(TB):
                    sel = selp.tile([P, P], BF16, tag="sel")
                    nc.vector.tensor_tensor(
                        out=sel[:],
                        in0=idx[:, TB + t : TB + t + 1].to_broadcast([P, P]),
                        in1=iota_sb[:],
                        op=OP.is_equal,
                    )
                    sels.append(sel)
                # one PSUM accumulation group at a time per bank region
                for fc in range(FC):
                    for t in range(TB):
                        nc.tensor.matmul(
                            out=pa[:, fc * P : (fc + 1) * P],
                            lhsT=msg[:, t * H + fc * P : t * H + (fc + 1) * P],
                            rhs=sels[t][:],
                            start=(t == 0), stop=(t == TB - 1),
                        )
                # h^T chunks: ht[:, fc*W + b*128 .. +128] = xt chunk + pa chunk
                ht_ap = bass.AP(ht.tensor, ht[:, 0].offset + b * P,
                                [ht[:, 0].ap[0], [W, FC], [1, P]])
                xt_ap = bass.AP(xt.tensor, xt[:, 0].offset + b * P,
                                [xt[:, 0].ap[0], [W, FC], [1, P]])
                pa_ap = bass.AP(pa.tensor, pa[:, 0].offset,
                                [pa[:, 0].ap[0], [P, FC], [1, P]])
                nc.vector.tensor_tensor(out=ht_ap, in0=xt_ap, in1=pa_ap, op=OP.add)
                if dbg and si == 0 and b == 0:
                    nc.sync.dma_start(out=dbg_xs[:], in_=xs[:])
                    nc.sync.dma_start(out=dbg_sel[0], in_=sels[0][:])
                    pacp = msgp.tile([P, FC * P], F32, tag="pacp")
                    nc.scalar.activation(out=pacp[:], in_=pa[:], func=AF.Copy)
                    nc.sync.dma_start(out=dbg_pa[:], in_=pacp[:])
            if dbg and si == 0:
                nc.sync.dma_start(out=dbg_ht[:], in_=ht[:])
            state[si] = (xt, ht)

        def phase_bcd(si):
            b0, nb = SBS[si]
            W = nb * P
            O = SBOFF[si]
            xt, ht = state.pop(si)
            # ---- MLP1 + gelu ----
            gt = gtp.tile([P, F2C * 4 * P], BF16, tag="gt")
            for f2c in range(F2C):
                pz = pzp.tile([P, 4 * P], F32, tag="pz")
                for kc in range(FC):
                    nc.tensor.matmul(
                        out=pz[:, :W],
                        lhsT=w1s[:, kc * H4 + f2c * P : kc * H4 + (f2c + 1) * P],
                        rhs=ht[:, kc * W : (kc + 1) * W],
                        start=(kc == 0), stop=(kc == FC - 1))
                act_fn = AF.Tanh if os.environ.get("KBD_ACT") == "tanh" else AF.Gelu
                nc.scalar.activation(out=gt[:, f2c * W : (f2c + 1) * W],
                                     in_=pz[:, :W], func=act_fn)
            # ---- MLP2 + residual ----
            ts = tsp.tile([P, FC * 4 * P], BF16, tag="ts")
            for fc in range(FC):
                py = pyp.tile([P, 4 * P], F32, tag="py")
                for kc in range(F2C):
                    nc.tensor.matmul(
                        out=py[:, :W],
                        lhsT=w2s[:, kc * H + fc * P : kc * H + (fc + 1) * P],
                        rhs=gt[:, kc * W : (kc + 1) * W],
                        start=(kc == 0), stop=(kc == F2C - 1))
                nc.vector.tensor_tensor(out=ts[:, fc * W : (fc + 1) * W],
                                        in0=xt[:, fc * W : (fc + 1) * W],
                                        in1=py[:, :W], op=OP.add)
            # ---- LayerNorm over features (partition axis via ones-matmul) ----
            ps = psp.tile([P, 4 * P], F32, tag="ps")
            for fc in range(FC):
                nc.tensor.matmul(out=ps[:, :W], lhsT=ones_sb[:],
                                 rhs=ts[:, fc * W : (fc + 1) * W],
                                 start=(fc == 0), stop=(fc == FC - 1))
            mean = stp.tile([P, 4 * P], F32, tag="mean")
            nc.vector.tensor_scalar_mul(out=mean[:, :W], in0=ps[:, :W], scalar1=inv_h)
            pq = psp.tile([P, 4 * P], F32, tag="ps")
            for fc in range(FC):
                sq = sqp.tile([P, 4 * P], BF16, tag="sq")
                nc.scalar.activation(out=sq[:, :W], in_=ts[:, fc * W : (fc + 1) * W],
                                     func=AF.Square)
                nc.tensor.matmul(out=pq[:, :W], lhsT=ones_sb[:], rhs=sq[:, :W],
                                 start=(fc == 0), stop=(fc == FC - 1))
            msq = stp.tile([P, 4 * P], F32, tag="tmp")
            nc.scalar.activation(out=msq[:, :W], in_=mean[:, :W], func=AF.Square)
            var = stp.tile([P, 4 * P], F32, tag="tmp")
            nc.vector.scalar_tensor_tensor(
                out=var[:, :W], in0=pq[:, :W], scalar=inv_h, in1=msq[:, :W],
                op0=OP.mult, op1=OP.subtract)
            std = stp.tile([P, 4 * P], F32, tag="tmp")
            nc.scalar.activation(out=std[:, :W], in_=var[:, :W], func=AF.Sqrt,
                                 bias=eps_sb[:])
            rstd = stp.tile([P, 4 * P], F32, tag="rstd")
            nc.vector.reciprocal(out=rstd[:, :W], in_=std[:, :W])
            o = outp.tile([P, FC * 4 * P], BF16, tag="o")
            for fc in range(FC):
                u = up.tile([P, 4 * P], F32, tag="u")
                nc.vector.tensor_tensor(out=u[:, :W], in0=ts[:, fc * W : (fc + 1) * W],
                                        in1=mean[:, :W], op=OP.subtract)
                if apply_gamma_beta:
                    nc.vector.scalar_tensor_tensor(
                        out=u[:, :W], in0=u[:, :W], scalar=gbt_sb[:, fc : fc + 1],
                        in1=rstd[:, :W], op0=OP.mult, op1=OP.mult)
                    nc.vector.tensor_scalar_add(
                        out=o[:, fc * W : (fc + 1) * W], in0=u[:, :W],
                        scalar1=gbt_sb[:, FC + fc : FC + fc + 1])
                else:
                    nc.vector.tensor_tensor(out=o[:, fc * W : (fc + 1) * W],
                                            in0=u[:, :W], in1=rstd[:, :W],
                                            op=OP.mult)
            nc.sync.dma_start(out=outD[:, O : O + FC * W], in_=o[:, : FC * W])

        phase_a(0)
        for si in range(len(SBS)):
            if si + 1 < len(SBS):
                phase_a(si + 1)
            phase_bcd(si)

    nc.compile()
    return nc


def _prep(x, edge_attr, W1, W2, gamma, beta, edge_index):
    src = np.asarray(edge_index[0], dtype=np.int64)
    dst = np.asarray(edge_index[1], dtype=np.int64)
    x = np.asarray(x, dtype=np.float32)
    edge_attr = np.asarray(edge_attr, dtype=np.float32)

    xb = x.astype(ml_dtypes.bfloat16)
    eab = edge_attr.astype(ml_dtypes.bfloat16)

    owner = dst // NLOC
    order = np.argsort(owner * N + dst, kind="stable")
    src_s, dst_s, eid_s = src[order], dst[order], order

    dstl = dst_s - owner[order] * NLOC          # local node id on owner core
    blk_of = dstl // P                          # local block id (0..NBLK-1)
    core_of = owner[order]
    counts = np.zeros((NC_, NBLK), dtype=np.int64)
    np.add.at(counts, (core_of, blk_of), 1)
    TB = max(1, int(np.ceil(counts.max() / P)))

    run_starts = np.zeros((NC_, NBLK), dtype=np.int64)
    run_starts.reshape(-1)[1:] = np.cumsum(counts.reshape(-1))[:-1]

    w1b = np.asarray(W1, dtype=np.float32).astype(ml_dtypes.bfloat16)
    w2b = np.asarray(W2, dtype=np.float32).astype(ml_dtypes.bfloat16)
    # w1p[p, kc*H4 + j] = W1[kc*128+p, j];  w2p[p, kc*H + f] = W2[kc*128+p, f]
    w1pk = np.ascontiguousarray(
        w1b.reshape(FC, P, H4).transpose(1, 0, 2).reshape(P, FC * H4))
    w2pk = np.ascontiguousarray(
        w2b.reshape(F2C, P, H).transpose(1, 0, 2).reshape(P, F2C * H))
    iota = np.broadcast_to(np.arange(P, dtype=np.int32), (P, P)).copy()
    gbtA = np.zeros((P, 2 * FC), dtype=np.float32)
    gbtA[:, :FC] = np.asarray(gamma, dtype=np.float32).reshape(FC, P).T
    gbtA[:, FC:] = np.asarray(beta, dtype=np.float32).reshape(FC, P).T

    in_maps = []
    for c in range(NC_):
        n0 = c * NLOC
        idxcomb = np.empty((NBLK, P, 2 * TB), dtype=np.int32)
        idxcomb[:, :, :TB] = OOB
        idxcomb[:, :, TB:] = -1
        eaid = np.zeros((NBLK, TB * P), dtype=np.int64)
        eamask = np.zeros((NBLK, TB * P), dtype=bool)
        for j in range(NBLK):
            cnt = int(counts[c, j])
            if cnt:
                s0 = int(run_starts[c, j])
                sl = slice(s0, s0 + cnt)
                kk = np.arange(cnt)
                t_i, p_i = kk // P, kk % P
                idxcomb[j, p_i, t_i] = src_s[sl].astype(np.int32)
                idxcomb[j, p_i, TB + t_i] = (dst_s[sl] - n0 - j * P).astype(np.int32)
                eaid[j, t_i * P + p_i] = eid_s[sl]
                eamask[j, t_i * P + p_i] = True
        for j in range(WARM):
            blk = idxcomb[j, :, :TB]
            blk[blk == OOB] = 0
        # eapack[j, p, t*H:(t+1)*H] = edge_attr[edge at slot (j,p,t)]
        eapk = np.zeros((NBLK, P, TB, H), dtype=ml_dtypes.bfloat16)
        flat = eapk.reshape(NBLK, P * TB, H)
        for j in range(NBLK):
            m = eamask[j]
            if m.any():
                ids = eaid[j][m]
                tt, pp = np.nonzero(m.reshape(TB, P))
                flat[j, pp * TB + tt] = eab[ids]
        eapk = flat.reshape(NBLK, P, TB * H)

        xloc = np.zeros((NLOCP, H), dtype=ml_dtypes.bfloat16)
        xloc[:NLOC] = xb[n0 : n0 + NLOC]
        # xtpack[p, O_sb + fc*W + n'] = xloc[sb_start + n', fc*128 + p]
        xtpk = np.empty((P, XCOLS), dtype=ml_dtypes.bfloat16)
        for si, (b0, nb) in enumerate(SBS):
            Wn = nb * P
            seg = xloc[b0 * P : b0 * P + Wn]           # [Wn, H]
            seg = seg.reshape(Wn, FC, P).transpose(2, 1, 0)  # [P, FC, Wn]
            xtpk[:, SBOFF[si] : SBOFF[si] + FC * Wn] = seg.reshape(P, FC * Wn)

        in_maps.append({
            "xfull": np.ascontiguousarray(xb), "idxcomb": idxcomb,
            "eapack": np.ascontiguousarray(eapk),
            "xtpack": np.ascontiguousarray(xtpk),
            "w1p": w1pk, "w2p": w2pk, "iota": iota, "gbt": gbtA,
        })
    return in_maps, TB


LAST_EXEC_NS = None


def kernel(x, edge_attr, W1, W2, gamma, beta, edge_index):
    global LAST_EXEC_NS
    in_maps, TB = _prep(x, edge_attr, W1, W2, gamma, beta, edge_index)
    gamma_np = np.asarray(gamma, dtype=np.float32)
    beta_np = np.asarray(beta, dtype=np.float32)
    apply_gb = not (np.all(gamma_np == 1.0) and np.all(beta_np == 0.0))
    nc = _build_program(TB, apply_gb)
    try:
        from concourse.timeline_sim import TimelineSim
        LAST_EXEC_NS = int(TimelineSim(nc, trace=False).simulate())
    except Exception:
        LAST_EXEC_NS = None
    rr = run_bass_kernel_spmd(nc, in_maps, list(range(NC_)))
    if rr.exec_time_ns is not None:
        LAST_EXEC_NS = int(rr.exec_time_ns)
    res = rr.results
    out = np.empty((N, H), dtype=np.float32)
    for c in range(NC_):
        od = np.asarray(res[c]["outD"], dtype=np.float32)   # [P, XCOLS]
        loc = np.empty((NLOCP, H), dtype=np.float32)
        for si, (b0, nb) in enumerate(SBS):
            Wn = nb * P
            seg = od[:, SBOFF[si] : SBOFF[si] + FC * Wn].reshape(P, FC, Wn)
            loc[b0 * P : b0 * P + Wn] = seg.transpose(2, 1, 0).reshape(Wn, H)
        out[c * NLOC : (c + 1) * NLOC] = loc[:NLOC]
    return out
